# revision 15
# baseline (speedup 1.0000x reference)
"""PersistentMemoryAttention Trainium2 kernel.

Sharding: 8 cores = 2 batches x 4 kv-heads (tensor parallel over kv heads,
data parallel over batch). Each core computes, for its (batch b, kv-head h):
  - q projection for its 4 query heads, k/v projection for its kv head
  - value-embedding gating, RoPE + QK rms-norm
  - persistent-memory-prefix GQA attention (causal over tokens)
  - output projection against its 256-column slice of Wproj (partial sum)
Host gathers by summing the 4 per-kv-head partial projections per batch.
"""

import sys

sys.path.insert(0, "/opt/trn_rl_repo")

import numpy as np

import concourse.bass as bass
import concourse.mybir as mybir
import concourse.tile as tile
from concourse import bacc, bass_utils
from concourse.bass import ts

F32 = mybir.dt.float32
F32R = mybir.dt.float32r
AX = mybir.AxisListType.X
AF = mybir.ActivationFunctionType

B, T, C = 2, 2048, 1024
NH, NKV, HD = 16, 4, 64
M = 64
GC = 32
EPS = 1e-6
P = 128
TT = T // P          # 16 T-tiles
KT = C // P          # 8 contraction tiles
NC2 = 4              # T-chunks of 512
CH = 512
SCORE_SCALE = float(1.2 * 1.2 / np.sqrt(np.float32(HD)))

N_CORES = 8

_compiled = None


def build_kernel(stage=3):
    nc = bacc.Bacc("TRN2", target_bir_lowering=False, debug=False,
                   enable_asserts=True, num_devices=N_CORES)

    # ---- DRAM I/O ----
    xT_d = nc.dram_tensor("xT", (C, T), F32R, kind="ExternalInput").ap()
    wqkv_d = nc.dram_tensor("wqkv", (C, 388), F32R, kind="ExternalInput").ap()
    ve_d = nc.dram_tensor("ve", (T, HD), F32, kind="ExternalInput").ap()
    cos_d = nc.dram_tensor("cosd", (T, 32), F32, kind="ExternalInput").ap()
    sin_d = nc.dram_tensor("sind", (T, 32), F32, kind="ExternalInput").ap()
    memk_d = nc.dram_tensor("memk", (M, HD), F32, kind="ExternalInput").ap()
    memv_d = nc.dram_tensor("memv", (M, HD), F32R, kind="ExternalInput").ap()
    vs_d = nc.dram_tensor("vs", (M, 1), F32, kind="ExternalInput").ap()
    wproj_d = nc.dram_tensor("wproj", (256, C), F32R, kind="ExternalInput").ap()
    trim_d = nc.dram_tensor("trim", (P, 512), F32R, kind="ExternalInput").ap()
    iden_d = nc.dram_tensor("iden", (P, P), F32, kind="ExternalInput").ap()
    out_d = nc.dram_tensor("out", (T, C), F32, kind="ExternalOutput").ap()

    with tile.TileContext(nc) as tc:
        with tc.tile_pool(name="persist", bufs=1) as pers:
            WQKV = pers.tile([P, KT, 388], F32R)
            WP = pers.tile([P, 2, C], F32R)
            COS = pers.tile([P, TT, 32], F32)
            SIN = pers.tile([P, TT, 32], F32)
            VE = pers.tile([P, TT, HD], F32)
            MEMK = pers.tile([M, HD], F32)
            MVAUG = pers.tile([M, HD + 1], F32R)
            VS = pers.tile([M, 1], F32)
            TRIM = pers.tile([P, 512], F32R)
            IDEN = pers.tile([P, P], F32)
            ONES = pers.tile([HD + 1, M], F32R)  # row 64 used (ones)
            EPSC = pers.tile([P, 1], F32)

            QT = pers.tile([HD, 4, T], F32R)            # q heads, transposed
            KTt = pers.tile([HD, M + T], F32R)          # mem ++ tokens, transposed
            VAUG = pers.tile([P, TT, HD + 1], F32R)     # v with trailing ones col
            YP = pers.tile([P, 2, T], F32R)             # packed y_att (4 heads)
            GS = pers.tile([P, TT], F32)

            nc.sync.dma_start(WQKV[:], wqkv_d.rearrange("(ko p) n -> p ko n", p=P))
            nc.sync.dma_start(WP[:], wproj_d.rearrange("(ko p) n -> p ko n", p=P))
            nc.sync.dma_start(COS[:], cos_d.rearrange("(n p) j -> p n j", p=P))
            nc.sync.dma_start(SIN[:], sin_d.rearrange("(n p) j -> p n j", p=P))
            nc.sync.dma_start(VE[:], ve_d.rearrange("(n p) d -> p n d", p=P))
            nc.sync.dma_start(MEMK[:], memk_d[:])
            nc.sync.dma_start(MVAUG[:, 0:HD], memv_d[:])
            nc.sync.dma_start(VS[:], vs_d[:])
            nc.sync.dma_start(TRIM[:], trim_d[:])
            nc.sync.dma_start(IDEN[:], iden_d[:])
            ONESF = pers.tile([P, M], F32)
            nc.vector.memset(ONESF[:], 1.0)
            nc.vector.memset(EPSC[:], EPS)
            nc.vector.tensor_copy(ONES[:], ONESF[0:HD + 1, :])
            nc.vector.tensor_copy(
                VAUG[:, :, HD:HD + 1],
                ONESF[:, 0:1].unsqueeze(1).to_broadcast([P, TT, 1]))
            nc.vector.tensor_copy(MVAUG[:, HD:HD + 1], ONESF[0:M, 0:1])
            # mem_v * v_scale
            nc.vector.tensor_scalar_mul(MVAUG[:, 0:HD], MVAUG[:, 0:HD], VS[:])

            # ================= phase 1: projections, rope, rms =================
            with tc.tile_pool(name="xpool", bufs=1) as xp, \
                 tc.tile_pool(name="ph1sb", bufs=3) as sb1, \
                 tc.tile_pool(name="vraw_p", bufs=1) as vrp, \
                 tc.tile_pool(name="ph1ps", bufs=2, space="PSUM") as ps1, \
                 tc.tile_pool(name="tps", bufs=4, space="PSUM") as pst:

                X = xp.tile([P, KT, T], F32R)
                nc.sync.dma_start(X[:], xT_d.rearrange("(ko p) t -> p ko t", p=P))

                VRAW = vrp.tile([P, TT, HD + 1], F32)

                # mem_k: rms-normalize, transpose into KTt[:, 0:M]
                msq = sb1.tile([M, HD], F32, tag="msq")
                nc.vector.tensor_mul(msq[:], MEMK[:], MEMK[:])
                msum = sb1.tile([M, 1], F32, tag="msum")
                nc.vector.reduce_sum(msum[:], msq[:], axis=AX)
                mrinv = sb1.tile([M, 1], F32, tag="mrinv")
                nc.scalar.activation(mrinv[:], msum[:], AF.Sqrt,
                                     bias=EPSC[0:M], scale=1.0 / HD)
                nc.vector.reciprocal(mrinv[:], mrinv[:])
                mkn = sb1.tile([M, HD], F32, tag="msq")
                nc.vector.tensor_mul(mkn[:], MEMK[:],
                                     mrinv[:].to_broadcast([M, HD]))
                ptm = pst.tile([HD, P], F32, tag="tp")
                nc.tensor.transpose(ptm[:, 0:M], mkn[:], IDEN[0:M, 0:M])
                nc.vector.tensor_copy(KTt[:, 0:M], ptm[:, 0:M])

                for i in range(TT):
                    pq = ps1.tile([P, 388], F32, tag="qkv")
                    for kt in range(KT):
                        nc.tensor.matmul(pq[:], X[:, kt, ts(i, P)],
                                         WQKV[:, kt, :],
                                         start=(kt == 0), stop=(kt == KT - 1))

                    R6 = pq[:, 0:384].rearrange("p (g d) -> p g d", d=HD)
                    q1 = R6[:, 0:5, 0:32]
                    q2 = R6[:, 0:5, 32:64]
                    cb = COS[:, i, :].unsqueeze(1).to_broadcast([P, 5, 32])
                    sbr = SIN[:, i, :].unsqueeze(1).to_broadcast([P, 5, 32])
                    ta = sb1.tile([P, 5, 32], F32, tag="ta")
                    tb = sb1.tile([P, 5, 32], F32, tag="tb")
                    qkr = sb1.tile([P, 5, HD], F32, tag="qkr")
                    nc.vector.tensor_mul(ta[:], q1, cb)
                    nc.vector.tensor_mul(tb[:], q2, sbr)
                    nc.vector.tensor_sub(qkr[:, :, 0:32], ta[:], tb[:])
                    nc.vector.tensor_mul(ta[:], q1, sbr)
                    nc.vector.tensor_mul(tb[:], q2, cb)
                    nc.vector.tensor_add(qkr[:, :, 32:64], ta[:], tb[:])
                    # rms: sum of squares over hd, rsqrt, scale
                    sq = sb1.tile([P, 5, HD], F32, tag="sq")
                    nc.vector.tensor_mul(sq[:], qkr[:], qkr[:])
                    sums = sb1.tile([P, 5], F32, tag="sums")
                    nc.vector.reduce_sum(sums[:], sq[:], axis=AX)
                    rinv = sb1.tile([P, 5], F32, tag="rinv")
                    nc.scalar.activation(rinv[:], sums[:], AF.Sqrt,
                                         bias=EPSC[:], scale=1.0 / HD)
                    nc.vector.reciprocal(rinv[:], rinv[:])
                    qkn = sb1.tile([P, 5, HD], F32, tag="qkn")
                    nc.vector.tensor_mul(
                        qkn[:], qkr[:],
                        rinv[:].unsqueeze(2).to_broadcast([P, 5, HD]))
                    # stash raw v + raw gate (psum slot is recycled later)
                    nc.vector.tensor_copy(VRAW[:, i], pq[:, 320:385])
                    # transposes into [hd, t] layouts
                    for hh in range(4):
                        pt = pst.tile([HD, P], F32, tag="tp")
                        nc.tensor.transpose(pt[:], qkn[:, hh, :], IDEN[:])
                        nc.vector.tensor_copy(QT[:, hh, ts(i, P)], pt[:])
                    pt = pst.tile([HD, P], F32, tag="tp")
                    nc.tensor.transpose(pt[:], qkn[:, 4, :], IDEN[:])
                    nc.vector.tensor_copy(KTt[:, M + i * P:M + (i + 1) * P], pt[:])

                # gates (single sigmoid call), then v gating
                nc.scalar.activation(GS[:], VRAW[:, :, HD], AF.Sigmoid)
                nc.vector.tensor_scalar_mul(GS[:], GS[:], 3.0)
                for i in range(TT):
                    tv = sb1.tile([P, HD], F32, tag="tv")
                    nc.vector.tensor_scalar_mul(tv[:], VE[:, i, :], GS[:, i:i + 1])
                    nc.vector.tensor_add(VAUG[:, i, 0:HD], tv[:],
                                         VRAW[:, i, 0:HD])

            if stage <= 1:
                nc.sync.dma_start(out_d[0:HD, 0:1024],
                                  KTt[:, 0:1024].bitcast(F32))
                nc.sync.dma_start(out_d[HD:2 * HD, 0:1024],
                                  QT[:, 0, 0:1024].bitcast(F32))
                nc.sync.dma_start(
                    out_d[P:P + P, 0:1024],
                    VAUG.bitcast(F32).rearrange("p a b -> p (a b)")[:, 0:1024])

            # ================= phase 2+3: attention + projection =================
            with tc.tile_pool(name="scps", bufs=2, space="PSUM") as scps, \
                 tc.tile_pool(name="yps", bufs=2, space="PSUM") as yps, \
                 tc.tile_pool(name="bps", bufs=1, space="PSUM") as bps, \
                 tc.tile_pool(name="prjps", bufs=1, space="PSUM") as prjps, \
                 tc.tile_pool(name="expp", bufs=3) as expp, \
                 tc.tile_pool(name="ph2sb", bufs=2) as sb2, \
                 tc.tile_pool(name="ph3sb", bufs=2) as sb3:

                for c in range(NC2 if stage >= 2 else 0):
                    n_tok = 4 * c + 4       # token S-tiles for this chunk
                    for h in range(4):
                        rhs_q = QT[:, h, ts(c, CH)]
                        py = yps.tile([P, CH], F32, tag="y")
                        # S-tiles: -1 = mem prefix, 1..n_tok = token tiles
                        stiles = [-1] + list(range(1, n_tok + 1))
                        pairs = [stiles[k:k + 2] for k in range(0, len(stiles), 2)]
                        n_pv = len(stiles)
                        pv_done = 0
                        for pair in pairs:
                            psc = scps.tile([P, 1024], F32, tag="sc")
                            for sub, j in enumerate(pair):
                                col = sub * CH
                                if j < 0:
                                    nc.tensor.matmul(psc[0:M, col:col + CH],
                                                     KTt[:, 0:M], rhs_q,
                                                     start=True, stop=True)
                                else:
                                    nc.tensor.matmul(
                                        psc[:, col:col + CH],
                                        KTt[:, M + (j - 1) * P:M + j * P],
                                        rhs_q, start=True, stop=True)
                            # exp (scale folds the 1.2*1.2/sqrt(hd))
                            ext = expp.tile([P, 1024], F32R, tag="ex")
                            if pair[0] < 0:
                                nc.scalar.activation(ext[0:M, 0:CH], psc[0:M, 0:CH],
                                                     AF.Exp, scale=SCORE_SCALE)
                                if len(pair) > 1:
                                    nc.scalar.activation(ext[:, CH:2 * CH],
                                                         psc[:, CH:2 * CH],
                                                         AF.Exp, scale=SCORE_SCALE)
                            else:
                                w = len(pair) * CH
                                nc.scalar.activation(ext[:, 0:w], psc[:, 0:w],
                                                     AF.Exp, scale=SCORE_SCALE)
                            # causal masks on diagonal-region tiles
                            for sub, j in enumerate(pair):
                                if j < 0:
                                    continue
                                rr = j - 4 * c
                                if rr >= 1:
                                    w = rr * P
                                    sl = ext[:, sub * CH:sub * CH + w]
                                    nc.vector.tensor_mul(
                                        sl, sl,
                                        TRIM[:, (4 - rr) * P:(4 - rr) * P + w])
                            # PV (+ softmax denominator via trailing ones col)
                            for sub, j in enumerate(pair):
                                col = sub * CH
                                pv_done += 1
                                last = pv_done == n_pv
                                if j < 0:
                                    nc.tensor.matmul(py[0:M + 1, :], MVAUG[:],
                                                     ext[0:M, 0:CH],
                                                     start=True, stop=last)
                                else:
                                    rr = j - 4 * c
                                    f0 = max(0, (rr - 1) * P)
                                    nc.tensor.matmul(
                                        py[0:HD + 1, f0:CH],
                                        VAUG[:, j - 1, :],
                                        ext[:, col + f0:col + CH],
                                        start=False, stop=last)
                        # normalize rows 0..63 by row 64 (softmax denominator)
                        ssb = sb2.tile([HD + 1, CH], F32R, tag="ss")
                        nc.vector.tensor_copy(ssb[HD:HD + 1, :], py[HD:HD + 1, :])
                        pb = bps.tile([HD, CH], F32, tag="bc")
                        nc.tensor.matmul(pb[:], ONES[HD:HD + 1, :],
                                         ssb[HD:HD + 1, :],
                                         start=True, stop=True)
                        inv = sb2.tile([HD, CH], F32, tag="inv")
                        nc.vector.reciprocal(inv[:], pb[:])
                        g = h // 2
                        if h % 2 == 0:
                            nc.vector.tensor_mul(YP[0:HD, g, ts(c, CH)],
                                                 py[0:HD, :], inv[:])
                        else:
                            tmp = sb2.tile([HD, CH], F32R, tag="tmp")
                            nc.vector.tensor_mul(tmp[:], py[0:HD, :], inv[:])
                            nc.sync.dma_start(YP[HD:P, g, ts(c, CH)], tmp[:])

                    # ---- output projection for this T-chunk ----
                    if stage <= 2:
                        continue
                    for it in range(4 * c, 4 * c + 4):
                        for n in range(2):
                            pp = prjps.tile([P, CH], F32, tag="pp")
                            for kt2 in range(2):
                                nc.tensor.matmul(pp[:], YP[:, kt2, ts(it, P)],
                                                 WP[:, kt2, ts(n, CH)],
                                                 start=(kt2 == 0), stop=(kt2 == 1))
                            ot = sb3.tile([P, CH], F32, tag="ot")
                            nc.vector.tensor_copy(ot[:], pp[:])
                            nc.sync.dma_start(out_d[ts(it, P), ts(n, CH)], ot[:])
                if stage == 2:
                    nc.sync.dma_start(out_d[0:P, 0:1024],
                                      YP[:, 0, 0:1024].bitcast(F32))

    nc.compile()
    return nc


def _make_in_maps(x, ve, cos, sin, Wq, Wk, Wv, Wproj, Wg, mem_k, mem_v, v_scale):
    f = np.float32
    cos = np.ascontiguousarray(cos, f)
    sin = np.ascontiguousarray(sin, f)
    trim = (np.arange(512)[None, :] - 384 >= np.arange(P)[:, None]).astype(f)
    iden = np.eye(P, dtype=f)
    vs_rep = np.full((M, 1), np.asarray(v_scale).reshape(-1)[0], f)
    in_maps = []
    for core in range(N_CORES):
        b, h = core // 4, core % 4
        xT = np.ascontiguousarray(x[b].T, f)
        gcol = np.zeros((4, C), f)
        gcol[0, :GC] = Wg[h]
        wqkv = np.ascontiguousarray(
            np.concatenate([Wq[256 * h:256 * h + 256],
                            Wk[64 * h:64 * h + 64],
                            Wv[64 * h:64 * h + 64],
                            gcol], 0).T, f)
        in_maps.append(dict(
            xT=xT,
            wqkv=wqkv,
            ve=np.ascontiguousarray(ve[b, :, 64 * h:64 * h + 64], f),
            cosd=cos, sind=sin,
            memk=np.ascontiguousarray(mem_k[0, :, h, :], f),
            memv=np.ascontiguousarray(mem_v[0, :, h, :], f),
            vs=vs_rep,
            wproj=np.ascontiguousarray(Wproj[:, 256 * h:256 * h + 256].T, f),
            trim=trim, iden=iden,
        ))
    return in_maps


def kernel(**inputs):
    global _compiled
    if _compiled is None:
        _compiled = build_kernel()
    in_maps = _make_in_maps(**inputs)
    res = bass_utils.run_bass_kernel_spmd(
        _compiled, in_maps, core_ids=list(range(N_CORES)))
    outs = [res.results[c]["out"] for c in range(N_CORES)]
    full = np.stack([
        outs[0] + outs[1] + outs[2] + outs[3],
        outs[4] + outs[5] + outs[6] + outs[7],
    ]).astype(np.float32)
    return full
